# revision 4
# baseline (speedup 1.0000x reference)
"""Trainium2 Bass kernel for nn_Attention_81020263072470 (v2).

Math (reference):
    q = queries @ W_q.T                    [B, H]
    k = keys @ W_k.T                       [B, S, H]
    scores = tanh(k + q[:, None, :]) @ w_v [B, S]
    e = exp(scores); att = e / sum(e)      (global sum over all B*S)
    rep = einsum('bsd,bs->bd', keys, att)  [B, D]

Data-parallel over batch (4 per core). Host passes keys pre-transposed
[b, d, s] bf16. Per core the S axis is processed in 32 tiles of 512,
grouped in PAIRS sharing one 2-bank PSUM tile:
  - pk2 [128, 2, 512] accumulates both tiles' k^T chunk for one hc,
    matmuls interleaved across the two banks (dc outer, half inner)
  - per-half ACT tanh [128, 512] with per-partition bias q^T
  - 8 score matmuls (w_v chunks, self-loading f32r) -> psc[half]
  - per-half ACT exp with fused row sum (esums)
  - per tile: Pool partition_broadcast of e, then 4 affine_mul_reduce
    DVE ops accumulate rep straight from bf16 keysT (+ add chain)
Unnormalized sums + partial exp-sums return to host; host divides.
Measured 147.5us HW exec (baseline 233.5us); rel err 1.53e-3.
"""
import numpy as np
from contextlib import ExitStack

# Fixed variant flags (see kernel_x.py bisection history):
#   amr: custom affine_mul_reduce DVE op -- the native TENSOR_TENSOR_REDUCE
#        ISA op hangs this hardware, so the weighted sum uses the proven
#        custom op plus an explicit add chain.
VARIANT = {"amr"}

# ---- problem constants (hardcoded per contract) ----
B, S, D, H = 32, 4096, 512, 512
N_CORES = 8
B_SHARD = B // N_CORES          # 4 batches per core
TILE_S = 512                    # s-rows per tile
N_TILES = S // TILE_S           # 8 tiles per batch
N_PAIRS = N_TILES // 2          # 4 pairs per batch
N_DC = D // 128                 # 4 d-chunks
N_HC = H // 128                 # 4 h-chunks
N_GROUPS = B_SHARD * N_PAIRS    # 16 pairs per core

_RUNNER = None
_NC = None


def _build_nc(repeat=1):
    import concourse.bacc as bacc
    import concourse.tile as tile
    import concourse.mybir as mybir

    F32 = mybir.dt.float32
    F32R = mybir.dt.float32r
    BF16 = mybir.dt.bfloat16
    AF = mybir.ActivationFunctionType
    ALU = mybir.AluOpType

    nc = bacc.Bacc("TRN2", target_bir_lowering=False, debug=False,
                   num_devices=N_CORES)

    kT_d = nc.dram_tensor("kT_s", [B_SHARD, N_TILES, 128, N_DC, TILE_S], BF16,
                          kind="ExternalInput")
    wkT_d = nc.dram_tensor("wkT", [128, N_DC, H], BF16, kind="ExternalInput")
    wvT_d = nc.dram_tensor("wvT", [128, N_HC], F32R, kind="ExternalInput")
    qT_d = nc.dram_tensor("qT", [128, N_HC, B_SHARD], F32, kind="ExternalInput")
    rep_d = nc.dram_tensor("rep_acc", [128, B_SHARD * N_DC], F32,
                           kind="ExternalOutput")
    esum_d = nc.dram_tensor("esums", [1, B_SHARD * N_TILES], F32,
                            kind="ExternalOutput")

    with ExitStack() as ctx:
        tc = ctx.enter_context(tile.TileContext(nc))
        cpool = ctx.enter_context(tc.tile_pool(name="const", bufs=1))
        p_kT = ctx.enter_context(tc.tile_pool(name="kT", bufs=8))
        p_tanh = ctx.enter_context(tc.tile_pool(name="tanh", bufs=3))
        p_small = ctx.enter_context(tc.tile_pool(name="small", bufs=3))
        ps = ctx.enter_context(tc.tile_pool(name="psum", bufs=1, space="PSUM"))

        wkT = cpool.tile([128, N_DC, H], BF16)
        nc.sync.dma_start(wkT[:], wkT_d[:])
        wvT = cpool.tile([128, N_HC], F32R)
        nc.sync.dma_start(wvT[:], wvT_d[:])
        qT = cpool.tile([128, N_HC, B_SHARD], F32)
        nc.sync.dma_start(qT[:], qT_d[:])

        esums = cpool.tile([1, B_SHARD * N_TILES], F32)
        rep_acc = cpool.tile([128, B_SHARD * N_DC], F32)

        def emit_pair_mms(b, p, kTs, tanh_sb, hcs, interject=None):
            """k-proj matmuls for pair p of batch b over the given hc list.
            kTs = (kT_even, kT_odd). Calls interject(idx) between hc groups."""
            for i, hc in enumerate(hcs):
                if "onebank" in VARIANT:
                    pkA = ps.tile([128, TILE_S], F32, tag="pk", bufs=4)
                    pkB = ps.tile([128, TILE_S], F32, tag="pk", bufs=4)
                    pks = (pkA, pkB)
                else:
                    pk2 = ps.tile([128, 2, TILE_S], F32, tag="pk", bufs=2)
                    pks = (pk2[:, 0], pk2[:, 1])
                if "contig" in VARIANT:
                    for half in range(2):
                        for dc in range(N_DC):
                            w = wkT[:, dc, hc * 128:(hc + 1) * 128]
                            nc.tensor.matmul(
                                pks[half], w, kTs[half][:, dc],
                                start=(dc == 0), stop=(dc == N_DC - 1))
                else:
                    for dc in range(N_DC):
                        w = wkT[:, dc, hc * 128:(hc + 1) * 128]
                        nc.tensor.matmul(
                            pks[0], w, kTs[0][:, dc],
                            start=(dc == 0), stop=(dc == N_DC - 1))
                        nc.tensor.matmul(
                            pks[1], w, kTs[1][:, dc],
                            start=(dc == 0), stop=(dc == N_DC - 1))
                if "xbank" in VARIANT and "onebank" not in VARIANT:
                    nc.scalar.activation(
                        tanh_sb[:, hc], pk2[:], AF.Tanh,
                        bias=qT[:, hc, b:b + 1])
                else:
                    for half in range(2):
                        nc.scalar.activation(
                            tanh_sb[:, hc, half * TILE_S:(half + 1) * TILE_S],
                            pks[half], AF.Tanh,
                            bias=qT[:, hc, b:b + 1])
                if interject is not None:
                    interject(i)

        def emit_tail(state, pscs):
            """exp + broadcast + weighted-sum for a finished pair."""
            b, p, kTs, tanh_sb = state
            for half in range(2):
                t = 2 * p + half
                ti = b * N_TILES + t
                e_sb = p_small.tile([1, TILE_S], BF16, tag="e")
                nc.scalar.activation(e_sb[:], pscs[half][:],
                                     AF.Exp,
                                     accum_out=esums[0:1, ti:ti + 1])
                e_rep = p_small.tile([128, TILE_S], BF16, tag="erep")
                nc.gpsimd.partition_broadcast(e_rep[:], e_sb[:])
                scr = p_small.tile([128, TILE_S], BF16, tag="scr")
                bc = b * N_DC
                if "amr" in VARIANT:
                    rep_t = p_small.tile([128, N_DC], F32, tag="rept")
                    for dc in range(N_DC):
                        nc.vector.affine_mul_reduce(
                            out=scr[:], accum_out=rep_t[:, dc:dc + 1],
                            in0=kTs[half][:, dc], in1=e_rep[:],
                            scale=1.0, bias=0.0)
                    if t == 0:
                        nc.vector.tensor_copy(
                            rep_acc[:, bc:bc + N_DC], rep_t[:])
                    else:
                        nc.vector.tensor_add(
                            rep_acc[:, bc:bc + N_DC],
                            rep_acc[:, bc:bc + N_DC], rep_t[:])
                else:
                    for dc in range(N_DC):
                        col = rep_acc[:, bc + dc:bc + dc + 1]
                        nc.vector.tensor_tensor_reduce(
                            out=scr[:], in0=kTs[half][:, dc], in1=e_rep[:],
                            scale=1.0,
                            scalar=(0.0 if t == 0 else col),
                            op0=ALU.mult, op1=ALU.add,
                            accum_out=col)

        def emit_scores(state, pscs):
            """8 score matmuls for a pair (accumulate over hc, one PSUM
            bank per half -- walrus only allows matmul dst partition 0)."""
            b, p, kTs, tanh_sb = state
            for hc in range(N_HC):
                for half in range(2):
                    nc.tensor.matmul(
                        pscs[half][:], wvT[:, hc:hc + 1],
                        tanh_sb[:, hc, half * TILE_S:(half + 1) * TILE_S],
                        start=(hc == 0), stop=(hc == N_HC - 1))

        for _rep in range(repeat):
            pending = None
            pending_psc = None
            for b in range(B_SHARD):
                for p in range(N_PAIRS):
                    kT0 = p_kT.tile([128, N_DC, TILE_S], BF16, tag="kt")
                    nc.sync.dma_start(kT0[:], kT_d[b, 2 * p])
                    kT1 = p_kT.tile([128, N_DC, TILE_S], BF16, tag="kt")
                    nc.sync.dma_start(kT1[:], kT_d[b, 2 * p + 1])
                    kTs = (kT0, kT1)
                    tanh_sb = p_tanh.tile([128, N_HC, 2 * TILE_S], F32R)
                    prev = pending
                    prev_psc = pending_psc

                    def interject(i, prev=prev, prev_psc=prev_psc):
                        if prev is None:
                            return
                        if i == 0:
                            emit_scores(prev, prev_psc)
                        elif i == 1:
                            emit_tail(prev, prev_psc)

                    emit_pair_mms(b, p, kTs, tanh_sb, range(N_HC),
                                  interject=interject)
                    pending = (b, p, kTs, tanh_sb)
                    psc0 = ps.tile([1, TILE_S], F32, tag="psc", bufs=4)
                    psc1 = ps.tile([1, TILE_S], F32, tag="psc", bufs=4)
                    pending_psc = (psc0, psc1)
            emit_scores(pending, pending_psc)
            emit_tail(pending, pending_psc)

        nc.sync.dma_start(rep_d[:], rep_acc[:])
        nc.sync.dma_start(esum_d[:], esums[:])

    nc.compile()
    return nc


def _make_runner(repeat=1):
    """Build the Bass module and return a jitted SPMD callable."""
    import jax
    import numpy as _np
    from jax.sharding import Mesh, PartitionSpec
    from jax.experimental.shard_map import shard_map
    import concourse.mybir as mybir
    from concourse import bass2jax

    bass2jax.install_neuronx_cc_hook()
    global _NC
    nc = _build_nc(repeat)
    if repeat == 1:
        _NC = nc

    partition_name = (nc.partition_id_tensor.name
                      if nc.partition_id_tensor else None)
    in_names, out_names, out_avals, zero_shapes = [], [], [], []
    for alloc in nc.m.functions[0].allocations:
        if not isinstance(alloc, mybir.MemoryLocationSet):
            continue
        name = alloc.memorylocations[0].name
        if alloc.kind == "ExternalInput":
            if name != partition_name:
                in_names.append(name)
        elif alloc.kind == "ExternalOutput":
            shape = tuple(alloc.tensor_shape)
            dtype = mybir.dt.np(alloc.dtype)
            out_names.append(name)
            out_avals.append(jax.core.ShapedArray(shape, dtype))
            zero_shapes.append((shape, dtype))
    n_params = len(in_names)
    all_in_names = list(in_names) + list(out_names)
    if partition_name is not None:
        all_in_names.append(partition_name)

    def _body(*args):
        operands = list(args)
        if partition_name is not None:
            operands.append(bass2jax.partition_id_tensor())
        outs = bass2jax._bass_exec_p.bind(
            *operands,
            out_avals=tuple(out_avals),
            in_names=tuple(all_in_names),
            out_names=tuple(out_names),
            lowering_input_output_aliases=(),
            sim_require_finite=True,
            sim_require_nnan=True,
            nc=nc,
        )
        return tuple(outs)

    devices = jax.devices()[:N_CORES]
    mesh = Mesh(_np.asarray(devices), ("core",))
    n_outs = len(out_names)
    in_specs = (PartitionSpec("core"),) * (n_params + n_outs)
    out_specs = (PartitionSpec("core"),) * n_outs
    sharded = jax.jit(
        shard_map(_body, mesh=mesh, in_specs=in_specs, out_specs=out_specs,
                  check_rep=False),
        donate_argnums=tuple(range(n_params, n_params + n_outs)),
        keep_unused=True,
    )

    def make_zeros():
        return [_np.zeros((N_CORES * s[0], *s[1:]), dt)
                for (s, dt) in zero_shapes]

    return sharded, in_names, out_names, make_zeros, mesh


def _get_runner():
    global _RUNNER
    if _RUNNER is None:
        _RUNNER = _make_runner()
    return _RUNNER


def _prep_inputs(keys, queries, W_k, W_q, w_v):
    """Host-side prep: shard keys, transform small tensors. Returns a dict
    name -> concatenated-along-axis-0 global array (per-core shards)."""
    import ml_dtypes
    BF = ml_dtypes.bfloat16

    keys = np.asarray(keys, dtype=np.float32)
    keysT = np.ascontiguousarray(
        keys.transpose(0, 2, 1)).astype(BF)          # [B, D, S]
    # tile-contiguous layout [B, t, p, dc, s]: elem = keysT[b, dc*128+p, t*T+s]
    keysT = np.ascontiguousarray(
        keysT.reshape(B, N_DC, 128, N_TILES, TILE_S).transpose(0, 3, 2, 1, 4))
    q = (queries.astype(np.float32) @ W_q.astype(np.float32).T)  # [B, H]

    # WkT host layout [128, dc, H]: [p, dc, h] = W_k[h, dc*128+p]
    wkT = np.ascontiguousarray(
        W_k.astype(np.float32).T.reshape(N_DC, 128, H).transpose(1, 0, 2)
    ).astype(BF)
    wvT = np.ascontiguousarray(w_v.astype(np.float32)[0].reshape(N_HC, 128).T)

    ins = {"kT_s": keysT,                         # [B, D, S] (axis0 -> 4/core)
           "wkT": np.tile(wkT, (N_CORES, 1, 1)),  # replicated
           "wvT": np.tile(wvT, (N_CORES, 1))}
    qT_all = []
    for c in range(N_CORES):
        qc = q[c * B_SHARD:(c + 1) * B_SHARD]     # [4, H]
        qT_all.append(qc.T.reshape(N_HC, 128, B_SHARD).transpose(1, 0, 2))
    ins["qT"] = np.ascontiguousarray(np.concatenate(qT_all, axis=0))
    return ins


def kernel(keys, queries, W_k, W_q, w_v):
    sharded, in_names, out_names, make_zeros, _mesh = _get_runner()
    ins = _prep_inputs(keys, queries, W_k, W_q, w_v)
    args = [ins[n] for n in in_names] + make_zeros()
    outs = sharded(*args)
    res = {n: np.asarray(outs[i]) for i, n in enumerate(out_names)}
    # rep_acc: per core [128, b*N_DC + dc] with d = dc*128 + p
    acc = res["rep_acc"].reshape(N_CORES, 128, B_SHARD, N_DC)
    rep_raw = acc.transpose(0, 2, 3, 1).reshape(B, D)   # [b, dc*128+p]
    esum_total = np.float32(res["esums"].astype(np.float64).sum())
    return (rep_raw / esum_total).astype(np.float32)


# revision 6
# speedup vs baseline: 1.1723x; 1.1723x over previous
"""Trainium2 Bass kernel for nn_Attention_81020263072470 (v2).

Math (reference):
    q = queries @ W_q.T                    [B, H]
    k = keys @ W_k.T                       [B, S, H]
    scores = tanh(k + q[:, None, :]) @ w_v [B, S]
    e = exp(scores); att = e / sum(e)      (global sum over all B*S)
    rep = einsum('bsd,bs->bd', keys, att)  [B, D]

Data-parallel over batch (4 per core). Host passes keys pre-transposed
[b, d, s] bf16. Per core the S axis is processed in 32 tiles of 512,
grouped in PAIRS sharing one 2-bank PSUM tile:
  - pk2 [128, 2, 512] accumulates both tiles' k^T chunk for one hc,
    matmuls interleaved across the two banks (dc outer, half inner)
  - per-half ACT tanh [128, 512] with per-partition bias q^T
  - 8 score matmuls (w_v chunks, self-loading f32r) -> psc[half]
  - per-half ACT exp with fused row sum (esums)
  - per tile: Pool partition_broadcast of e, then 4 affine_mul_reduce
    DVE ops accumulate rep straight from bf16 keysT (+ add chain)
Unnormalized sums + partial exp-sums return to host; host divides.
fp8 hybrid: h-chunks 0-1 of the k-projection run as fp8e4 DoubleRow
matmuls (K=256 each) from an fp8 copy of keysT; h-chunks 2-3 and the
weighted sum stay bf16. Measured 142.7us HW exec (bf16-only variant
147.5-167us, baseline 233.5us); rel err 1.433e-2 (deterministic,
reproduced across three independent HW runs; gate 2e-2).
"""
import numpy as np
from contextlib import ExitStack

# Fixed variant flags (see kernel_x.py bisection history):
#   amr: custom affine_mul_reduce DVE op -- the native TENSOR_TENSOR_REDUCE
#        ISA op hangs this hardware, so the weighted sum uses the proven
#        custom op plus an explicit add chain.
VARIANT = {"amr"}

# fp8 hybrid: h-chunks [0, N_HC8) of the k-projection run in fp8e4 with
# DoubleRow perf mode (K=256 per matmul, ~1.5x PE throughput); the rest
# stay bf16. Measured numpy rel err at N_HC8=2: 1.43e-2 (< 2e-2 gate).
N_HC8 = 2

# ---- problem constants (hardcoded per contract) ----
B, S, D, H = 32, 4096, 512, 512
N_CORES = 8
B_SHARD = B // N_CORES          # 4 batches per core
TILE_S = 512                    # s-rows per tile
N_TILES = S // TILE_S           # 8 tiles per batch
N_PAIRS = N_TILES // 2          # 4 pairs per batch
N_DC = D // 128                 # 4 d-chunks
N_HC = H // 128                 # 4 h-chunks
N_GROUPS = B_SHARD * N_PAIRS    # 16 pairs per core

_RUNNER = None
_NC = None


def _build_nc(repeat=1):
    import concourse.bacc as bacc
    import concourse.tile as tile
    import concourse.mybir as mybir

    F32 = mybir.dt.float32
    F32R = mybir.dt.float32r
    BF16 = mybir.dt.bfloat16
    FP8 = mybir.dt.float8e4
    AF = mybir.ActivationFunctionType
    ALU = mybir.AluOpType
    DR = mybir.MatmulPerfMode.DoubleRow

    nc = bacc.Bacc("TRN2", target_bir_lowering=False, debug=False,
                   num_devices=N_CORES)

    kT_d = nc.dram_tensor("kT_s", [B_SHARD, N_TILES, 128, N_DC, TILE_S], BF16,
                          kind="ExternalInput")
    # fp8 copy of keysT packed for DoubleRow: d = dc2*256 + i*128 + p
    k8_d = nc.dram_tensor("k8_s", [B_SHARD, N_TILES, 128, N_DC // 2, 2,
                                   TILE_S], FP8, kind="ExternalInput")
    # fp8 W_k rows [0, 128*N_HC8) packed for DoubleRow
    w8_d = nc.dram_tensor("w8T", [128, N_DC // 2, 2, N_HC8 * 128], FP8,
                          kind="ExternalInput")
    wkT_d = nc.dram_tensor("wkT", [128, N_DC, H], BF16, kind="ExternalInput")
    wvT_d = nc.dram_tensor("wvT", [128, N_HC], F32R, kind="ExternalInput")
    qT_d = nc.dram_tensor("qT", [128, N_HC, B_SHARD], F32, kind="ExternalInput")
    rep_d = nc.dram_tensor("rep_acc", [128, B_SHARD * N_DC], F32,
                           kind="ExternalOutput")
    esum_d = nc.dram_tensor("esums", [1, B_SHARD * N_TILES], F32,
                            kind="ExternalOutput")

    with ExitStack() as ctx:
        tc = ctx.enter_context(tile.TileContext(nc))
        cpool = ctx.enter_context(tc.tile_pool(name="const", bufs=1))
        p_kT = ctx.enter_context(tc.tile_pool(name="kT", bufs=8))
        p_tanh = ctx.enter_context(tc.tile_pool(name="tanh", bufs=3))
        p_small = ctx.enter_context(tc.tile_pool(name="small", bufs=3))
        ps = ctx.enter_context(tc.tile_pool(name="psum", bufs=1, space="PSUM"))

        wkT = cpool.tile([128, N_DC, H], BF16)
        nc.sync.dma_start(wkT[:], wkT_d[:])
        w8T = cpool.tile([128, N_DC // 2, 2, N_HC8 * 128], FP8)
        nc.sync.dma_start(w8T[:], w8_d[:])
        wvT = cpool.tile([128, N_HC], F32R)
        nc.sync.dma_start(wvT[:], wvT_d[:])
        qT = cpool.tile([128, N_HC, B_SHARD], F32)
        nc.sync.dma_start(qT[:], qT_d[:])

        esums = cpool.tile([1, B_SHARD * N_TILES], F32)
        rep_acc = cpool.tile([128, B_SHARD * N_DC], F32)

        def emit_pair_mms(b, p, kTs, k8s, tanh_sb, hcs, interject=None):
            """k-proj matmuls for pair p of batch b over the given hc list.
            kTs = (kT_even, kT_odd) bf16; k8s likewise fp8-DoubleRow-packed.
            hc < N_HC8 run as fp8 DoubleRow (K=256/mm); the rest bf16."""
            for i, hc in enumerate(hcs):
                pk2 = ps.tile([128, 2, TILE_S], F32, tag="pk", bufs=2)
                pks = (pk2[:, 0], pk2[:, 1])
                if hc < N_HC8:
                    nd2 = N_DC // 2
                    for dc2 in range(nd2):
                        w8 = w8T[:, dc2, :, hc * 128:(hc + 1) * 128]
                        nc.tensor.matmul(
                            pks[0], w8, k8s[0][:, dc2],
                            start=(dc2 == 0), stop=(dc2 == nd2 - 1),
                            perf_mode=DR)
                        nc.tensor.matmul(
                            pks[1], w8, k8s[1][:, dc2],
                            start=(dc2 == 0), stop=(dc2 == nd2 - 1),
                            perf_mode=DR)
                else:
                    for dc in range(N_DC):
                        w = wkT[:, dc, hc * 128:(hc + 1) * 128]
                        nc.tensor.matmul(
                            pks[0], w, kTs[0][:, dc],
                            start=(dc == 0), stop=(dc == N_DC - 1))
                        nc.tensor.matmul(
                            pks[1], w, kTs[1][:, dc],
                            start=(dc == 0), stop=(dc == N_DC - 1))
                for half in range(2):
                    nc.scalar.activation(
                        tanh_sb[:, hc, half * TILE_S:(half + 1) * TILE_S],
                        pks[half], AF.Tanh,
                        bias=qT[:, hc, b:b + 1])
                if interject is not None:
                    interject(i)

        def emit_tail(state, pscs):
            """exp + broadcast + weighted-sum for a finished pair."""
            b, p, kTs, tanh_sb = state
            for half in range(2):
                t = 2 * p + half
                ti = b * N_TILES + t
                e_sb = p_small.tile([1, TILE_S], BF16, tag="e")
                nc.scalar.activation(e_sb[:], pscs[half][:],
                                     AF.Exp,
                                     accum_out=esums[0:1, ti:ti + 1])
                e_rep = p_small.tile([128, TILE_S], BF16, tag="erep")
                nc.gpsimd.partition_broadcast(e_rep[:], e_sb[:])
                scr = p_small.tile([128, TILE_S], BF16, tag="scr")
                bc = b * N_DC
                if "amr" in VARIANT:
                    rep_t = p_small.tile([128, N_DC], F32, tag="rept")
                    for dc in range(N_DC):
                        nc.vector.affine_mul_reduce(
                            out=scr[:], accum_out=rep_t[:, dc:dc + 1],
                            in0=kTs[half][:, dc], in1=e_rep[:],
                            scale=1.0, bias=0.0)
                    if t == 0:
                        nc.vector.tensor_copy(
                            rep_acc[:, bc:bc + N_DC], rep_t[:])
                    else:
                        nc.vector.tensor_add(
                            rep_acc[:, bc:bc + N_DC],
                            rep_acc[:, bc:bc + N_DC], rep_t[:])
                else:
                    for dc in range(N_DC):
                        col = rep_acc[:, bc + dc:bc + dc + 1]
                        nc.vector.tensor_tensor_reduce(
                            out=scr[:], in0=kTs[half][:, dc], in1=e_rep[:],
                            scale=1.0,
                            scalar=(0.0 if t == 0 else col),
                            op0=ALU.mult, op1=ALU.add,
                            accum_out=col)

        def emit_scores(state, pscs):
            """8 score matmuls for a pair (accumulate over hc, one PSUM
            bank per half -- walrus only allows matmul dst partition 0)."""
            b, p, kTs, tanh_sb = state
            for hc in range(N_HC):
                for half in range(2):
                    nc.tensor.matmul(
                        pscs[half][:], wvT[:, hc:hc + 1],
                        tanh_sb[:, hc, half * TILE_S:(half + 1) * TILE_S],
                        start=(hc == 0), stop=(hc == N_HC - 1))

        for _rep in range(repeat):
            pending = None
            pending_psc = None
            for b in range(B_SHARD):
                for p in range(N_PAIRS):
                    kT0 = p_kT.tile([128, N_DC, TILE_S], BF16, tag="kt")
                    nc.sync.dma_start(kT0[:], kT_d[b, 2 * p])
                    kT1 = p_kT.tile([128, N_DC, TILE_S], BF16, tag="kt")
                    nc.sync.dma_start(kT1[:], kT_d[b, 2 * p + 1])
                    kTs = (kT0, kT1)
                    k80 = p_kT.tile([128, N_DC // 2, 2, TILE_S], FP8,
                                    tag="k8", bufs=8)
                    nc.sync.dma_start(k80[:], k8_d[b, 2 * p])
                    k81 = p_kT.tile([128, N_DC // 2, 2, TILE_S], FP8,
                                    tag="k8", bufs=8)
                    nc.sync.dma_start(k81[:], k8_d[b, 2 * p + 1])
                    k8s = (k80, k81)
                    tanh_sb = p_tanh.tile([128, N_HC, 2 * TILE_S], F32R)
                    prev = pending
                    prev_psc = pending_psc

                    def interject(i, prev=prev, prev_psc=prev_psc):
                        if prev is None:
                            return
                        if i == 0:
                            emit_scores(prev, prev_psc)
                        elif i == 1:
                            emit_tail(prev, prev_psc)

                    emit_pair_mms(b, p, kTs, k8s, tanh_sb, range(N_HC),
                                  interject=interject)
                    pending = (b, p, kTs, tanh_sb)
                    psc0 = ps.tile([1, TILE_S], F32, tag="psc", bufs=4)
                    psc1 = ps.tile([1, TILE_S], F32, tag="psc", bufs=4)
                    pending_psc = (psc0, psc1)
            emit_scores(pending, pending_psc)
            emit_tail(pending, pending_psc)

        nc.sync.dma_start(rep_d[:], rep_acc[:])
        nc.sync.dma_start(esum_d[:], esums[:])

    nc.compile()
    return nc


def _make_runner(repeat=1):
    """Build the Bass module and return a jitted SPMD callable."""
    import jax
    import numpy as _np
    from jax.sharding import Mesh, PartitionSpec
    from jax.experimental.shard_map import shard_map
    import concourse.mybir as mybir
    from concourse import bass2jax

    bass2jax.install_neuronx_cc_hook()
    global _NC
    nc = _build_nc(repeat)
    if repeat == 1:
        _NC = nc

    partition_name = (nc.partition_id_tensor.name
                      if nc.partition_id_tensor else None)
    in_names, out_names, out_avals, zero_shapes = [], [], [], []
    for alloc in nc.m.functions[0].allocations:
        if not isinstance(alloc, mybir.MemoryLocationSet):
            continue
        name = alloc.memorylocations[0].name
        if alloc.kind == "ExternalInput":
            if name != partition_name:
                in_names.append(name)
        elif alloc.kind == "ExternalOutput":
            shape = tuple(alloc.tensor_shape)
            dtype = mybir.dt.np(alloc.dtype)
            out_names.append(name)
            out_avals.append(jax.core.ShapedArray(shape, dtype))
            zero_shapes.append((shape, dtype))
    n_params = len(in_names)
    all_in_names = list(in_names) + list(out_names)
    if partition_name is not None:
        all_in_names.append(partition_name)

    def _body(*args):
        operands = list(args)
        if partition_name is not None:
            operands.append(bass2jax.partition_id_tensor())
        outs = bass2jax._bass_exec_p.bind(
            *operands,
            out_avals=tuple(out_avals),
            in_names=tuple(all_in_names),
            out_names=tuple(out_names),
            lowering_input_output_aliases=(),
            sim_require_finite=True,
            sim_require_nnan=True,
            nc=nc,
        )
        return tuple(outs)

    devices = jax.devices()[:N_CORES]
    mesh = Mesh(_np.asarray(devices), ("core",))
    n_outs = len(out_names)
    in_specs = (PartitionSpec("core"),) * (n_params + n_outs)
    out_specs = (PartitionSpec("core"),) * n_outs
    sharded = jax.jit(
        shard_map(_body, mesh=mesh, in_specs=in_specs, out_specs=out_specs,
                  check_rep=False),
        donate_argnums=tuple(range(n_params, n_params + n_outs)),
        keep_unused=True,
    )

    def make_zeros():
        return [_np.zeros((N_CORES * s[0], *s[1:]), dt)
                for (s, dt) in zero_shapes]

    return sharded, in_names, out_names, make_zeros, mesh


def _get_runner():
    global _RUNNER
    if _RUNNER is None:
        _RUNNER = _make_runner()
    return _RUNNER


def _prep_inputs(keys, queries, W_k, W_q, w_v):
    """Host-side prep: shard keys, transform small tensors. Returns a dict
    name -> concatenated-along-axis-0 global array (per-core shards)."""
    import ml_dtypes
    BF = ml_dtypes.bfloat16
    F8 = ml_dtypes.float8_e4m3

    keys = np.asarray(keys, dtype=np.float32)
    keysT32 = np.ascontiguousarray(keys.transpose(0, 2, 1))   # [B, D, S] f32
    keysT = keysT32.astype(BF)
    # tile-contiguous layout [B, t, p, dc, s]: elem = keysT[b, dc*128+p, t*T+s]
    keysT = np.ascontiguousarray(
        keysT.reshape(B, N_DC, 128, N_TILES, TILE_S).transpose(0, 3, 2, 1, 4))
    # fp8 DoubleRow packing [B, t, p, dc2, i, s]: d = dc2*256 + i*128 + p
    keys8 = keysT32.astype(F8).reshape(
        B, N_DC // 2, 2, 128, N_TILES, TILE_S).transpose(0, 4, 3, 1, 2, 5)
    keys8 = np.ascontiguousarray(keys8)
    q = (queries.astype(np.float32) @ W_q.astype(np.float32).T)  # [B, H]

    # WkT host layout [128, dc, H]: [p, dc, h] = W_k[h, dc*128+p]
    wkT = np.ascontiguousarray(
        W_k.astype(np.float32).T.reshape(N_DC, 128, H).transpose(1, 0, 2)
    ).astype(BF)
    # fp8 W_k rows [0, N_HC8*128): [p, dc2, i, h] = W_k[h, dc2*256+i*128+p]
    w8T = np.ascontiguousarray(
        W_k[:N_HC8 * 128].astype(np.float32).T.reshape(
            N_DC // 2, 2, 128, N_HC8 * 128).transpose(2, 0, 1, 3)).astype(F8)
    wvT = np.ascontiguousarray(w_v.astype(np.float32)[0].reshape(N_HC, 128).T)

    ins = {"kT_s": keysT,                         # [B, D, S] (axis0 -> 4/core)
           "k8_s": keys8,
           "wkT": np.tile(wkT, (N_CORES, 1, 1)),  # replicated
           "w8T": np.tile(w8T, (N_CORES, 1, 1, 1)),
           "wvT": np.tile(wvT, (N_CORES, 1))}
    qT_all = []
    for c in range(N_CORES):
        qc = q[c * B_SHARD:(c + 1) * B_SHARD]     # [4, H]
        qT_all.append(qc.T.reshape(N_HC, 128, B_SHARD).transpose(1, 0, 2))
    ins["qT"] = np.ascontiguousarray(np.concatenate(qT_all, axis=0))
    return ins


def kernel(keys, queries, W_k, W_q, w_v):
    sharded, in_names, out_names, make_zeros, _mesh = _get_runner()
    ins = _prep_inputs(keys, queries, W_k, W_q, w_v)
    args = [ins[n] for n in in_names] + make_zeros()
    outs = sharded(*args)
    res = {n: np.asarray(outs[i]) for i, n in enumerate(out_names)}
    # rep_acc: per core [128, b*N_DC + dc] with d = dc*128 + p
    acc = res["rep_acc"].reshape(N_CORES, 128, B_SHARD, N_DC)
    rep_raw = acc.transpose(0, 2, 3, 1).reshape(B, D)   # [b, dc*128+p]
    esum_total = np.float32(res["esums"].astype(np.float64).sum())
    return (rep_raw / esum_total).astype(np.float32)


# revision 8
# speedup vs baseline: 1.2403x; 1.0580x over previous
"""Trainium2 Bass kernel for nn_Attention_81020263072470 (v2).

Math (reference):
    q = queries @ W_q.T                    [B, H]
    k = keys @ W_k.T                       [B, S, H]
    scores = tanh(k + q[:, None, :]) @ w_v [B, S]
    e = exp(scores); att = e / sum(e)      (global sum over all B*S)
    rep = einsum('bsd,bs->bd', keys, att)  [B, D]

Data-parallel over batch (4 per core). Host passes keys pre-transposed
[b, d, s] bf16. Per core the S axis is processed in 32 tiles of 512,
grouped in PAIRS sharing one 2-bank PSUM tile:
  - pk2 [128, 2, 512] accumulates both tiles' k^T chunk for one hc,
    matmuls interleaved across the two banks (dc outer, half inner)
  - per-half ACT tanh [128, 512] with per-partition bias q^T
  - 8 score matmuls (w_v chunks, self-loading f32r) -> psc[half]
  - per-half ACT exp with fused row sum (esums)
  - per tile: Pool partition_broadcast of e, then 4 affine_mul_reduce
    DVE ops accumulate rep straight from bf16 keysT (+ add chain)
Unnormalized sums + partial exp-sums return to host; host divides.
fp8 hybrid: h-chunks 0-1 of the k-projection run as fp8e4 DoubleRow
matmuls (K=256 each) from an fp8 copy of keysT; h-chunks 2-3 and the
weighted sum stay bf16. PSUM: pk pairs triple-buffered (6 banks) +
psc double-buffered (2 banks) -- the extra pk slack hides the ACT
tanh drain behind the shorter fp8 matmul groups.
Measured 127.7us HW exec (fp8 with pk bufs=2: 142.7us, bf16-only:
147.5-167us, baseline: 233.5us); rel err 1.433e-2 (deterministic
across HW runs; gate 2e-2).
"""
import numpy as np
from contextlib import ExitStack

# Fixed variant flags (see kernel_x.py bisection history):
#   amr: custom affine_mul_reduce DVE op -- the native TENSOR_TENSOR_REDUCE
#        ISA op hangs this hardware, so the weighted sum uses the proven
#        custom op plus an explicit add chain.
VARIANT = {"amr"}

# fp8 hybrid: h-chunks [0, N_HC8) of the k-projection run in fp8e4 with
# DoubleRow perf mode (K=256 per matmul, ~1.5x PE throughput); the rest
# stay bf16. Measured numpy rel err at N_HC8=2: 1.43e-2 (< 2e-2 gate).
N_HC8 = 2

# ---- problem constants (hardcoded per contract) ----
B, S, D, H = 32, 4096, 512, 512
N_CORES = 8
B_SHARD = B // N_CORES          # 4 batches per core
TILE_S = 512                    # s-rows per tile
N_TILES = S // TILE_S           # 8 tiles per batch
N_PAIRS = N_TILES // 2          # 4 pairs per batch
N_DC = D // 128                 # 4 d-chunks
N_HC = H // 128                 # 4 h-chunks
N_GROUPS = B_SHARD * N_PAIRS    # 16 pairs per core

_RUNNER = None
_NC = None


def _build_nc(repeat=1):
    import concourse.bacc as bacc
    import concourse.tile as tile
    import concourse.mybir as mybir

    F32 = mybir.dt.float32
    F32R = mybir.dt.float32r
    BF16 = mybir.dt.bfloat16
    FP8 = mybir.dt.float8e4
    AF = mybir.ActivationFunctionType
    ALU = mybir.AluOpType
    DR = mybir.MatmulPerfMode.DoubleRow

    nc = bacc.Bacc("TRN2", target_bir_lowering=False, debug=False,
                   num_devices=N_CORES)

    kT_d = nc.dram_tensor("kT_s", [B_SHARD, N_TILES, 128, N_DC, TILE_S], BF16,
                          kind="ExternalInput")
    # fp8 copy of keysT packed for DoubleRow: d = dc2*256 + i*128 + p
    k8_d = nc.dram_tensor("k8_s", [B_SHARD, N_TILES, 128, N_DC // 2, 2,
                                   TILE_S], FP8, kind="ExternalInput")
    # fp8 W_k rows [0, 128*N_HC8) packed for DoubleRow
    w8_d = nc.dram_tensor("w8T", [128, N_DC // 2, 2, N_HC8 * 128], FP8,
                          kind="ExternalInput")
    wkT_d = nc.dram_tensor("wkT", [128, N_DC, H], BF16, kind="ExternalInput")
    wvT_d = nc.dram_tensor("wvT", [128, N_HC], F32R, kind="ExternalInput")
    qT_d = nc.dram_tensor("qT", [128, N_HC, B_SHARD], F32, kind="ExternalInput")
    rep_d = nc.dram_tensor("rep_acc", [128, B_SHARD * N_DC], F32,
                           kind="ExternalOutput")
    esum_d = nc.dram_tensor("esums", [1, B_SHARD * N_TILES], F32,
                            kind="ExternalOutput")

    with ExitStack() as ctx:
        tc = ctx.enter_context(tile.TileContext(nc))
        cpool = ctx.enter_context(tc.tile_pool(name="const", bufs=1))
        p_kT = ctx.enter_context(tc.tile_pool(name="kT", bufs=8))
        p_tanh = ctx.enter_context(tc.tile_pool(name="tanh", bufs=3))
        p_small = ctx.enter_context(tc.tile_pool(name="small", bufs=3))
        ps = ctx.enter_context(tc.tile_pool(name="psum", bufs=1, space="PSUM"))

        wkT = cpool.tile([128, N_DC, H], BF16)
        nc.sync.dma_start(wkT[:], wkT_d[:])
        w8T = cpool.tile([128, N_DC // 2, 2, N_HC8 * 128], FP8)
        nc.sync.dma_start(w8T[:], w8_d[:])
        wvT = cpool.tile([128, N_HC], F32R)
        nc.sync.dma_start(wvT[:], wvT_d[:])
        qT = cpool.tile([128, N_HC, B_SHARD], F32)
        nc.sync.dma_start(qT[:], qT_d[:])

        esums = cpool.tile([1, B_SHARD * N_TILES], F32)
        rep_acc = cpool.tile([128, B_SHARD * N_DC], F32)

        def emit_pair_mms(b, p, kTs, k8s, tanh_sb, hcs, interject=None):
            """k-proj matmuls for pair p of batch b over the given hc list.
            kTs = (kT_even, kT_odd) bf16; k8s likewise fp8-DoubleRow-packed.
            hc < N_HC8 run as fp8 DoubleRow (K=256/mm); the rest bf16."""
            for i, hc in enumerate(hcs):
                pk2 = ps.tile([128, 2, TILE_S], F32, tag="pk", bufs=3)
                pks = (pk2[:, 0], pk2[:, 1])
                if hc < N_HC8:
                    nd2 = N_DC // 2
                    for dc2 in range(nd2):
                        w8 = w8T[:, dc2, :, hc * 128:(hc + 1) * 128]
                        nc.tensor.matmul(
                            pks[0], w8, k8s[0][:, dc2],
                            start=(dc2 == 0), stop=(dc2 == nd2 - 1),
                            perf_mode=DR)
                        nc.tensor.matmul(
                            pks[1], w8, k8s[1][:, dc2],
                            start=(dc2 == 0), stop=(dc2 == nd2 - 1),
                            perf_mode=DR)
                else:
                    for dc in range(N_DC):
                        w = wkT[:, dc, hc * 128:(hc + 1) * 128]
                        nc.tensor.matmul(
                            pks[0], w, kTs[0][:, dc],
                            start=(dc == 0), stop=(dc == N_DC - 1))
                        nc.tensor.matmul(
                            pks[1], w, kTs[1][:, dc],
                            start=(dc == 0), stop=(dc == N_DC - 1))
                for half in range(2):
                    nc.scalar.activation(
                        tanh_sb[:, hc, half * TILE_S:(half + 1) * TILE_S],
                        pks[half], AF.Tanh,
                        bias=qT[:, hc, b:b + 1])
                if interject is not None:
                    interject(i)

        def emit_tail(state, pscs):
            """exp + broadcast + weighted-sum for a finished pair."""
            b, p, kTs, tanh_sb = state
            for half in range(2):
                t = 2 * p + half
                ti = b * N_TILES + t
                e_sb = p_small.tile([1, TILE_S], BF16, tag="e")
                nc.scalar.activation(e_sb[:], pscs[half][:],
                                     AF.Exp,
                                     accum_out=esums[0:1, ti:ti + 1])
                e_rep = p_small.tile([128, TILE_S], BF16, tag="erep")
                nc.gpsimd.partition_broadcast(e_rep[:], e_sb[:])
                scr = p_small.tile([128, TILE_S], BF16, tag="scr")
                bc = b * N_DC
                if "amr" in VARIANT:
                    rep_t = p_small.tile([128, N_DC], F32, tag="rept")
                    for dc in range(N_DC):
                        nc.vector.affine_mul_reduce(
                            out=scr[:], accum_out=rep_t[:, dc:dc + 1],
                            in0=kTs[half][:, dc], in1=e_rep[:],
                            scale=1.0, bias=0.0)
                    if t == 0:
                        nc.vector.tensor_copy(
                            rep_acc[:, bc:bc + N_DC], rep_t[:])
                    else:
                        nc.vector.tensor_add(
                            rep_acc[:, bc:bc + N_DC],
                            rep_acc[:, bc:bc + N_DC], rep_t[:])
                else:
                    for dc in range(N_DC):
                        col = rep_acc[:, bc + dc:bc + dc + 1]
                        nc.vector.tensor_tensor_reduce(
                            out=scr[:], in0=kTs[half][:, dc], in1=e_rep[:],
                            scale=1.0,
                            scalar=(0.0 if t == 0 else col),
                            op0=ALU.mult, op1=ALU.add,
                            accum_out=col)

        def emit_scores(state, pscs):
            """8 score matmuls for a pair (accumulate over hc, one PSUM
            bank per half -- walrus only allows matmul dst partition 0)."""
            b, p, kTs, tanh_sb = state
            for hc in range(N_HC):
                for half in range(2):
                    nc.tensor.matmul(
                        pscs[half][:], wvT[:, hc:hc + 1],
                        tanh_sb[:, hc, half * TILE_S:(half + 1) * TILE_S],
                        start=(hc == 0), stop=(hc == N_HC - 1))

        for _rep in range(repeat):
            pending = None
            pending_psc = None
            for b in range(B_SHARD):
                for p in range(N_PAIRS):
                    kT0 = p_kT.tile([128, N_DC, TILE_S], BF16, tag="kt")
                    nc.sync.dma_start(kT0[:], kT_d[b, 2 * p])
                    kT1 = p_kT.tile([128, N_DC, TILE_S], BF16, tag="kt")
                    nc.sync.dma_start(kT1[:], kT_d[b, 2 * p + 1])
                    kTs = (kT0, kT1)
                    k80 = p_kT.tile([128, N_DC // 2, 2, TILE_S], FP8,
                                    tag="k8", bufs=8)
                    nc.sync.dma_start(k80[:], k8_d[b, 2 * p])
                    k81 = p_kT.tile([128, N_DC // 2, 2, TILE_S], FP8,
                                    tag="k8", bufs=8)
                    nc.sync.dma_start(k81[:], k8_d[b, 2 * p + 1])
                    k8s = (k80, k81)
                    tanh_sb = p_tanh.tile([128, N_HC, 2 * TILE_S], F32R)
                    prev = pending
                    prev_psc = pending_psc

                    def interject(i, prev=prev, prev_psc=prev_psc):
                        if prev is None:
                            return
                        if i == 0:
                            emit_scores(prev, prev_psc)
                        elif i == 1:
                            emit_tail(prev, prev_psc)

                    emit_pair_mms(b, p, kTs, k8s, tanh_sb, range(N_HC),
                                  interject=interject)
                    pending = (b, p, kTs, tanh_sb)
                    psc0 = ps.tile([1, TILE_S], F32, tag="psc", bufs=2)
                    psc1 = ps.tile([1, TILE_S], F32, tag="psc", bufs=2)
                    pending_psc = (psc0, psc1)
            emit_scores(pending, pending_psc)
            emit_tail(pending, pending_psc)

        nc.sync.dma_start(rep_d[:], rep_acc[:])
        nc.sync.dma_start(esum_d[:], esums[:])

    nc.compile()
    return nc


def _make_runner(repeat=1):
    """Build the Bass module and return a jitted SPMD callable."""
    import jax
    import numpy as _np
    from jax.sharding import Mesh, PartitionSpec
    from jax.experimental.shard_map import shard_map
    import concourse.mybir as mybir
    from concourse import bass2jax

    bass2jax.install_neuronx_cc_hook()
    global _NC
    nc = _build_nc(repeat)
    if repeat == 1:
        _NC = nc

    partition_name = (nc.partition_id_tensor.name
                      if nc.partition_id_tensor else None)
    in_names, out_names, out_avals, zero_shapes = [], [], [], []
    for alloc in nc.m.functions[0].allocations:
        if not isinstance(alloc, mybir.MemoryLocationSet):
            continue
        name = alloc.memorylocations[0].name
        if alloc.kind == "ExternalInput":
            if name != partition_name:
                in_names.append(name)
        elif alloc.kind == "ExternalOutput":
            shape = tuple(alloc.tensor_shape)
            dtype = mybir.dt.np(alloc.dtype)
            out_names.append(name)
            out_avals.append(jax.core.ShapedArray(shape, dtype))
            zero_shapes.append((shape, dtype))
    n_params = len(in_names)
    all_in_names = list(in_names) + list(out_names)
    if partition_name is not None:
        all_in_names.append(partition_name)

    def _body(*args):
        operands = list(args)
        if partition_name is not None:
            operands.append(bass2jax.partition_id_tensor())
        outs = bass2jax._bass_exec_p.bind(
            *operands,
            out_avals=tuple(out_avals),
            in_names=tuple(all_in_names),
            out_names=tuple(out_names),
            lowering_input_output_aliases=(),
            sim_require_finite=True,
            sim_require_nnan=True,
            nc=nc,
        )
        return tuple(outs)

    devices = jax.devices()[:N_CORES]
    mesh = Mesh(_np.asarray(devices), ("core",))
    n_outs = len(out_names)
    in_specs = (PartitionSpec("core"),) * (n_params + n_outs)
    out_specs = (PartitionSpec("core"),) * n_outs
    sharded = jax.jit(
        shard_map(_body, mesh=mesh, in_specs=in_specs, out_specs=out_specs,
                  check_rep=False),
        donate_argnums=tuple(range(n_params, n_params + n_outs)),
        keep_unused=True,
    )

    def make_zeros():
        return [_np.zeros((N_CORES * s[0], *s[1:]), dt)
                for (s, dt) in zero_shapes]

    return sharded, in_names, out_names, make_zeros, mesh


def _get_runner():
    global _RUNNER
    if _RUNNER is None:
        _RUNNER = _make_runner()
    return _RUNNER


def _prep_inputs(keys, queries, W_k, W_q, w_v):
    """Host-side prep: shard keys, transform small tensors. Returns a dict
    name -> concatenated-along-axis-0 global array (per-core shards)."""
    import ml_dtypes
    BF = ml_dtypes.bfloat16
    F8 = ml_dtypes.float8_e4m3

    keys = np.asarray(keys, dtype=np.float32)
    keysT32 = np.ascontiguousarray(keys.transpose(0, 2, 1))   # [B, D, S] f32
    keysT = keysT32.astype(BF)
    # tile-contiguous layout [B, t, p, dc, s]: elem = keysT[b, dc*128+p, t*T+s]
    keysT = np.ascontiguousarray(
        keysT.reshape(B, N_DC, 128, N_TILES, TILE_S).transpose(0, 3, 2, 1, 4))
    # fp8 DoubleRow packing [B, t, p, dc2, i, s]: d = dc2*256 + i*128 + p
    keys8 = keysT32.astype(F8).reshape(
        B, N_DC // 2, 2, 128, N_TILES, TILE_S).transpose(0, 4, 3, 1, 2, 5)
    keys8 = np.ascontiguousarray(keys8)
    q = (queries.astype(np.float32) @ W_q.astype(np.float32).T)  # [B, H]

    # WkT host layout [128, dc, H]: [p, dc, h] = W_k[h, dc*128+p]
    wkT = np.ascontiguousarray(
        W_k.astype(np.float32).T.reshape(N_DC, 128, H).transpose(1, 0, 2)
    ).astype(BF)
    # fp8 W_k rows [0, N_HC8*128): [p, dc2, i, h] = W_k[h, dc2*256+i*128+p]
    w8T = np.ascontiguousarray(
        W_k[:N_HC8 * 128].astype(np.float32).T.reshape(
            N_DC // 2, 2, 128, N_HC8 * 128).transpose(2, 0, 1, 3)).astype(F8)
    wvT = np.ascontiguousarray(w_v.astype(np.float32)[0].reshape(N_HC, 128).T)

    ins = {"kT_s": keysT,                         # [B, D, S] (axis0 -> 4/core)
           "k8_s": keys8,
           "wkT": np.tile(wkT, (N_CORES, 1, 1)),  # replicated
           "w8T": np.tile(w8T, (N_CORES, 1, 1, 1)),
           "wvT": np.tile(wvT, (N_CORES, 1))}
    qT_all = []
    for c in range(N_CORES):
        qc = q[c * B_SHARD:(c + 1) * B_SHARD]     # [4, H]
        qT_all.append(qc.T.reshape(N_HC, 128, B_SHARD).transpose(1, 0, 2))
    ins["qT"] = np.ascontiguousarray(np.concatenate(qT_all, axis=0))
    return ins


def kernel(keys, queries, W_k, W_q, w_v):
    sharded, in_names, out_names, make_zeros, _mesh = _get_runner()
    ins = _prep_inputs(keys, queries, W_k, W_q, w_v)
    args = [ins[n] for n in in_names] + make_zeros()
    outs = sharded(*args)
    res = {n: np.asarray(outs[i]) for i, n in enumerate(out_names)}
    # rep_acc: per core [128, b*N_DC + dc] with d = dc*128 + p
    acc = res["rep_acc"].reshape(N_CORES, 128, B_SHARD, N_DC)
    rep_raw = acc.transpose(0, 2, 3, 1).reshape(B, D)   # [b, dc*128+p]
    esum_total = np.float32(res["esums"].astype(np.float64).sum())
    return (rep_raw / esum_total).astype(np.float32)
